# revision 24
# baseline (speedup 1.0000x reference)
"""Octonion-structured causal self-attention on 8 Trainium2 NeuronCores.

Strategy (2 SPMD launches, no collectives):
  Launch 1 — tensor-parallel over output-channel blocks (= 4 heads/core):
    each core computes q^T,k^T (RoPE'd, channel-pair-permuted) and v for its
    512-channel block from the full x^T, then causal attention for its 4
    heads, producing UNNORMALIZED y^T [512, 2048] bf16 plus the per-head
    softmax denominators d [4, 2048] f32 (normalization is a host divide —
    keeps the device TensorE queue free of reciprocal/broadcast stalls).
  Host — gathers y^T and d, normalizes, re-tiles; folds the octonion head
    mixer into Wo (both linear: Wfused = blockdiag(M_g) @ Weff_o).
  Launch 2 — tensor-parallel over output channels: each core computes
    out[:, c*512:(c+1)*512] = y_norm @ Wfused[:, c*512:(c+1)*512] by
    streaming y^T tiles against a resident 4MB weight slice.

All matmuls run in bf16 (TensorE full rate); accumulation is fp32 in PSUM.
Causal masking is fine-grained: diagonal 128-blocks use partial-width
matmuls plus a single [128,128] triangular mask multiply.

RoPE trick: channels of q/k are permuted host-side (per head: evens then
odds) by permuting W_q/W_k columns, so the rotation pairs become the two
partition halves of each head tile; scores are invariant to a shared q/k
channel permutation, and v/y stay in natural order.
"""
import json
import math
import sys

sys.path.insert(0, '/opt/trn_rl_repo')

import numpy as np
import ml_dtypes

import concourse.bass as bass
import concourse.mybir as mybir
import concourse.tile as tile

F32 = mybir.dt.float32
BF16 = mybir.dt.bfloat16
BF = ml_dtypes.bfloat16

B, T, C, H, D = 1, 2048, 4096, 32, 128
NC = 8            # cores
CPB = C // NC     # channels per core (512) = 4 heads
TBLK = 512        # projection T-block
TQB = 512         # attention query block
NTB = T // TBLK   # 4
INV_SQRT_D = 1.0 / math.sqrt(D)

# ---------------------------------------------------------------- walrus fix
# This container's walrus encodes at most ONE sync-wait per instruction;
# Tile attaches several. Split extras into single-wait NoOps just before the
# instruction (same engine => same program point; semantics unchanged).
_ws_counter = [0]


def _split_multiwaits_json(bir_bytes):
    m = json.loads(bir_bytes)
    changed_any = False
    for fn in m.get("functions", []):
        for blk in fn.get("blocks", []):
            insts = blk.get("instructions")
            if not insts:
                continue
            out, changed = [], False
            for inst in insts:
                si = inst.get("sync_info")
                waits = si.get("on_wait") if si else None
                if waits and len(waits) > 1:
                    changed = True
                    for w in waits[:-1]:
                        _ws_counter[0] += 1
                        out.append({
                            "engine": inst["engine"], "ins": [], "outs": [],
                            "name": f"I-wsplit-{_ws_counter[0]}",
                            "opcode": "NoOp",
                            "sync_info": {"on_wait": [w], "on_update": []},
                        })
                    si["on_wait"] = [waits[-1]]
                out.append(inst)
            if changed:
                blk["instructions"] = out
                changed_any = True
    return json.dumps(m).encode() if changed_any else bir_bytes


_patched = [False]


def _install_patch():
    if _patched[0]:
        return
    _patched[0] = True
    import concourse.bass_utils as bass_utils
    import concourse.bass2jax as bass2jax

    orig = bass_utils.compile_bir_kernel

    def patched(bir_json, tmpdir, neff_name="file.neff"):
        if isinstance(bir_json, str):
            bir_json = bir_json.encode()
        return orig(_split_multiwaits_json(bir_json), tmpdir, neff_name=neff_name)

    bass_utils.compile_bir_kernel = patched
    bass2jax.compile_bir_kernel = patched


# ------------------------------------------------------- octonion structure
def _cd_tables(levels=3):
    idx = np.array([[0]])
    sgn = np.array([[1]])
    for _ in range(levels):
        n = idx.shape[0]
        N2 = 2 * n
        I = np.zeros((N2, N2), np.int64)
        S = np.zeros((N2, N2), np.int64)
        cj = lambda j: 1 if j == 0 else -1
        for i in range(n):
            for j in range(n):
                I[i, j] = idx[i, j]
                S[i, j] = sgn[i, j]
                I[i, n + j] = n + idx[j, i]
                S[i, n + j] = sgn[j, i]
                I[n + i, j] = n + idx[i, j]
                S[n + i, j] = sgn[i, j] * cj(j)
                I[n + i, n + j] = idx[j, i]
                S[n + i, n + j] = -cj(j) * sgn[j, i]
        idx, sgn = I, S
    return idx, sgn


_OIDX, _OSGN = _cd_tables()
_SIGN = np.array([[_OSGN[j, i ^ j] for j in range(8)] for i in range(8)], np.float32)


def _weff(W):
    """[8, 512, 512] -> dense [4096, 4096]: block (row j, col i) = SIGN[i,j]*W[i^j]."""
    out = np.empty((C, C), np.float32)
    for i in range(8):
        for j in range(8):
            out[j * 512:(j + 1) * 512, i * 512:(i + 1) * 512] = _SIGN[i, j] * W[i ^ j]
    return out


# ----------------------------------------------------------- phase-1 kernel
def _build_phase1(reps=1):
    nc = bass.Bass(trn_type="TRN2")
    xt_d = nc.dram_tensor("xt", [NTB, 128, 32 * TBLK], BF16, kind="ExternalInput")
    wq_d = nc.dram_tensor("wq", [512, 4096], BF16, kind="ExternalInput")
    wk_d = nc.dram_tensor("wk", [512, 4096], BF16, kind="ExternalInput")
    wv_d = nc.dram_tensor("wv", [128, 32 * 512], BF16, kind="ExternalInput")
    cs_d = nc.dram_tensor("cs", [128, T], F32, kind="ExternalInput")
    sn_d = nc.dram_tensor("sn", [128, T], F32, kind="ExternalInput")
    tri_d = nc.dram_tensor("tri", [128, 128], BF16, kind="ExternalInput")
    yt_d = nc.dram_tensor("yt", [CPB, T], BF16, kind="ExternalOutput")
    dd_d = nc.dram_tensor("dd", [1, 4 * T], F32, kind="ExternalOutput")

    with tile.TileContext(nc) as tc:
        with tc.tile_pool(name="const", bufs=1) as constp, \
             tc.tile_pool(name="qkv", bufs=1) as qkvp, \
             tc.tile_pool(name="xres", bufs=2) as xp:

            # resident tiles (DMAs emitted later, after the first prefetches)
            qt_s = qkvp.tile([128, 4 * T], BF16, tag="qt")   # head h at [:, h*T:]
            kt_s = qkvp.tile([128, 4 * T], BF16, tag="kt")
            v_s = qkvp.tile([128, 16 * 512], BF16, tag="v")  # t-chunk tt at [:, tt*512:]

            consts_emitted = [False]
            cs_s = constp.tile([128, T], F32, tag="cs")
            sn_s = constp.tile([128, T], F32, tag="sn")
            tri_s = constp.tile([128, 128], BF16, tag="tri")
            wv_s = constp.tile([128, 32 * 512], BF16, tag="wv")
            ones_col = constp.tile([128, 1], BF16, tag="ones_col")
            warm = constp.tile([1, 2], F32, tag="warm")

            def emit_consts():
                consts_emitted[0] = True
                nc.sync.dma_start(cs_s[:], cs_d[:])
                nc.sync.dma_start(sn_s[:], sn_d[:])
                nc.sync.dma_start(tri_s[:], tri_d[:])
                nc.sync.dma_start(wv_s[:], wv_d[:])
                nc.any.memset(ones_col[:], 1.0)
                # prewarm the Exp activation table while DMAs stream
                nc.any.memset(warm[:], 0.0)
                nc.scalar.activation(warm[:], warm[:],
                                     mybir.ActivationFunctionType.Exp)

            for _rep in range(reps):
                # One fused section: attention blocks are interleaved into
                # the projection stream as soon as their (tqb <= tb-1) data
                # is ready, so the Scalar exp work spreads across the whole
                # kernel instead of binding a pure-attention tail.
                with tc.tile_pool(name="wqk", bufs=3) as wqkp, \
                     tc.tile_pool(name="ropet", bufs=2) as rtp, \
                     tc.tile_pool(name="ptile", bufs=6) as ptp, \
                     tc.tile_pool(name="yout", bufs=3) as osp, \
                     tc.tile_pool(name="dsb", bufs=2) as dsp, \
                     tc.tile_pool(name="ppsum", bufs=2, space="PSUM") as pps, \
                     tc.tile_pool(name="vpsum", bufs=1, space="PSUM") as vps, \
                     tc.tile_pool(name="spsum", bufs=2, space="PSUM") as sps, \
                     tc.tile_pool(name="ypsum", bufs=2, space="PSUM") as yps, \
                     tc.tile_pool(name="dpsum", bufs=1, space="PSUM") as dps:

                    # ---- attention emitter (software pipelined)
                    stream = [(h, tqb, tkb)
                              for tqb in range(4) for h in range(4)
                              for tkb in range(tqb * 4 + 4)]
                    LOOKAHEAD = 4
                    iters = {}
                    pend = []
                    pos = [0]

                    def emit_front(h, tqb, tkb):
                        if tkb == 0:
                            iters[(h, tqb)] = (
                                yps.tile([128, TQB], F32, tag="y",
                                         name=f"psy{_rep}_{h}_{tqb}"),
                                dps.tile([1, TQB], F32, tag="d",
                                         name=f"psd{_rep}_{h}_{tqb}"))
                        di = tkb - tqb * 4
                        q0 = di * 128 if di > 0 else 0
                        pss = sps.tile([128, TQB], F32, tag="s")
                        nc.tensor.matmul(
                            pss[:, q0:TQB],
                            kt_s[:, h * T + tkb * 128:h * T + (tkb + 1) * 128],
                            qt_s[:, h * T + tqb * TQB + q0:
                                 h * T + (tqb + 1) * TQB],
                            start=True, stop=True)
                        pt = ptp.tile([128, TQB], BF16, tag="p")
                        nc.scalar.activation(
                            pt[:, q0:TQB], pss[:, q0:TQB],
                            mybir.ActivationFunctionType.Exp, scale=INV_SQRT_D)
                        if di >= 0:
                            nc.vector.tensor_mul(
                                pt[:, q0:q0 + 128], pt[:, q0:q0 + 128],
                                tri_s[:])
                        return (h, tqb, tkb, q0, pt)

                    def emit_back(h, tqb, tkb, q0, pt):
                        psy, psd = iters[(h, tqb)]
                        nblk = tqb * 4 + 4
                        first, last = (tkb == 0), (tkb == nblk - 1)
                        nc.tensor.matmul(psd[:, q0:TQB], ones_col[:],
                                         pt[:, q0:TQB], start=first, stop=last)
                        nc.tensor.matmul(
                            psy[:, q0:TQB],
                            v_s[:, tkb * 512 + h * 128:
                                tkb * 512 + (h + 1) * 128],
                            pt[:, q0:TQB], start=first, stop=last)
                        if last:
                            db = dsp.tile([1, TQB], F32, tag="db")
                            nc.vector.tensor_copy(db[:], psd[:])
                            nc.sync.dma_start(
                                dd_d[0:1, h * T + tqb * TQB:
                                     h * T + (tqb + 1) * TQB], db[:])
                            yb = osp.tile([128, TQB], BF16, tag="yb")
                            nc.vector.tensor_copy(yb[:], psy[:])
                            nc.sync.dma_start(
                                yt_d[h * 128:(h + 1) * 128,
                                     tqb * TQB:(tqb + 1) * TQB], yb[:])
                            del iters[(h, tqb)]

                    def pump(n, max_tqb):
                        while (n > 0 and pos[0] < len(stream)
                               and stream[pos[0]][1] <= max_tqb):
                            pend.append(emit_front(*stream[pos[0]]))
                            pos[0] += 1
                            if len(pend) > LOOKAHEAD:
                                emit_back(*pend.pop(0))
                            n -= 1

                    def pump_rest():
                        while pos[0] < len(stream):
                            pend.append(emit_front(*stream[pos[0]]))
                            pos[0] += 1
                            if len(pend) > LOOKAHEAD:
                                emit_back(*pend.pop(0))
                        while pend:
                            emit_back(*pend.pop(0))

                    # ---- projection (attention pumped between groups)
                    strip_specs = [(colb, which)
                                   for colb in range(4) for which in range(2)]

                    def issue_strip(idx):
                        colb, which = strip_specs[idx % 8]
                        t = wqkp.tile([128, 4096], BF16, tag="w")
                        src = wq_d if which == 0 else wk_d
                        nc.sync.dma_start(t[:], src[colb * 128:(colb + 1) * 128, :])
                        return t

                    def issue_x(tb):
                        t = xp.tile([128, 32 * TBLK], BF16, tag="x")
                        nc.sync.dma_start(t[:], xt_d[tb, :, :])
                        return t

                    # prefetch: first x quarter + first weight strip ahead of
                    # the rest, so the first matmul starts ~6us in
                    x_cur = xp.tile([128, 32 * TBLK], BF16, tag="x")
                    xq = (32 * TBLK) // 4
                    w0 = wqkp.tile([128, 4096], BF16, tag="w", name=f"w0_{_rep}")
                    nc.sync.dma_start(x_cur[:, 0:xq], xt_d[0, :, 0:xq])
                    nc.sync.dma_start(w0[:, 0:1024], wq_d[0:128, 0:1024])
                    for s in range(1, 4):
                        nc.sync.dma_start(w0[:, s * 1024:(s + 1) * 1024],
                                          wq_d[0:128, s * 1024:(s + 1) * 1024])
                        nc.sync.dma_start(x_cur[:, s * xq:(s + 1) * xq],
                                          xt_d[0, :, s * xq:(s + 1) * xq])
                    pending = [w0, issue_strip(1), issue_strip(2)]
                    si = 3
                    if not consts_emitted[0]:
                        emit_consts()

                    def rope_epilogue(ps, tb, dst):
                        # q' = q*cos_full + swap(q)*sin_signed ; dst bf16
                        qsw = rtp.tile([128, TBLK], F32, tag="qsw")
                        nc.scalar.copy(qsw[0:64, :], ps[64:128, :])
                        nc.scalar.copy(qsw[64:128, :], ps[0:64, :])
                        t1 = rtp.tile([128, TBLK], F32, tag="t1")
                        nc.vector.tensor_mul(t1[:], ps[:],
                                             cs_s[:, tb * TBLK:(tb + 1) * TBLK])
                        nc.vector.tensor_mul(qsw[:], qsw[:],
                                             sn_s[:, tb * TBLK:(tb + 1) * TBLK])
                        nc.vector.tensor_add(dst, t1[:], qsw[:])

                    PUMP = [0, 2, 3, 4]
                    x_next = None
                    for tb in range(NTB):
                        for colb in range(4):
                            for which in range(2):   # 0 = q, 1 = k
                                w_s = pending.pop(0)
                                if si < 8 * NTB:
                                    pending.append(issue_strip(si))
                                    si += 1
                                ps = pps.tile([128, TBLK], F32, tag="pq")
                                for c in range(32):
                                    nc.tensor.matmul(
                                        ps[:], w_s[:, c * 128:(c + 1) * 128],
                                        x_cur[:, c * TBLK:(c + 1) * TBLK],
                                        start=(c == 0), stop=(c == 31))
                                dst_s = qt_s if which == 0 else kt_s
                                rope_epilogue(
                                    ps, tb,
                                    dst_s[:, colb * T + tb * TBLK:
                                          colb * T + (tb + 1) * TBLK])
                                pump(PUMP[tb], tb - 1)
                            if colb == 1 and tb < NTB - 1:
                                x_next = issue_x(tb + 1)

                        for t128 in range(4):
                            psv = vps.tile([128, 512], F32, tag="pv")
                            for c in range(32):
                                nc.tensor.matmul(
                                    psv[:],
                                    x_cur[:, c * TBLK + t128 * 128:
                                          c * TBLK + (t128 + 1) * 128],
                                    wv_s[:, c * 512:(c + 1) * 512],
                                    start=(c == 0), stop=(c == 31))
                            tt = tb * 4 + t128
                            nc.vector.tensor_copy(
                                v_s[:, tt * 512:(tt + 1) * 512], psv[:])
                            pump(PUMP[tb], tb - 1)
                        x_cur = x_next

                    pump_rest()
    return nc


# ----------------------------------------------------------- phase-2 kernel
def _build_phase2(reps=1):
    # out[:, core slice] = y_norm @ Wfused[:, core slice], c(ontraction)-outer:
    # 8 PSUM banks hold out row-tiles for half of T; y^T chunks stream per
    # contraction block so compute starts ~2us in (no full-weight wait).
    nc = bass.Bass(trn_type="TRN2")
    yt2_d = nc.dram_tensor("yt2", [32, 128, T], BF16, kind="ExternalInput")
    wf_d = nc.dram_tensor("wf", [128, 32 * 512], BF16, kind="ExternalInput")
    out_d = nc.dram_tensor("out", [T, 512], BF16, kind="ExternalOutput")
    HT = T // 2  # 1024 t-cols per pass

    with tile.TileContext(nc) as tc:
        with tc.tile_pool(name="wf", bufs=1) as wfp, \
             tc.tile_pool(name="yin", bufs=6) as inp, \
             tc.tile_pool(name="opsum", bufs=8, space="PSUM") as ops, \
             tc.tile_pool(name="osb", bufs=4) as osp:

            wf_s = wfp.tile([128, 32 * 512], BF16, tag="wf")
            wfi = [0]

            def issue_wf(k=1):  # stream the weight slice in c-chunks
                for _ in range(k):
                    if wfi[0] < 32:
                        c = wfi[0]
                        nc.sync.dma_start(wf_s[:, c * 512:(c + 1) * 512],
                                          wf_d[:, c * 512:(c + 1) * 512])
                        wfi[0] += 1

            yqi = [0]

            def issue_y(ph, c):
                t = inp.tile([128, HT], BF16, tag="y")
                yqi[0] += 1
                nc.sync.dma_start(t[:], yt2_d[c, :, ph * HT:(ph + 1) * HT])
                return t

            # stream y tiles with depth-3 lookahead to ride out DMA jitter
            seq = []
            for _rep in range(reps):
                for ph in range(2):
                    seq += [(ph, c) for c in range(32)]
            ytiles = [issue_y(*seq[0]), issue_y(*seq[1])]
            issue_wf(2)
            sqi = [2]

            def y_advance():
                if sqi[0] < len(seq):
                    ytiles.append(issue_y(*seq[sqi[0]]))
                    sqi[0] += 1
                return ytiles.pop(0)

            for _rep in range(reps):
                for ph in range(2):
                    pso = [ops.tile([128, 512], F32, tag="o",
                                    name=f"pso{_rep}_{ph}_{j}")
                           for j in range(8)]
                    for c in range(32):
                        y_cur = y_advance()
                        issue_wf(2)
                        for j in range(8):
                            nc.tensor.matmul(
                                pso[j][:],
                                y_cur[:, j * 128:(j + 1) * 128],
                                wf_s[:, c * 512:(c + 1) * 512],
                                start=(c == 0), stop=(c == 31))
                    for j in range(8):
                        tt = ph * 8 + j
                        ob = osp.tile([128, 512], BF16, tag="ob")
                        if j % 2 == 0:
                            nc.scalar.copy(ob[:], pso[j][:])
                        else:
                            nc.vector.tensor_copy(ob[:], pso[j][:])
                        nc.sync.dma_start(out_d[tt * 128:(tt + 1) * 128, :],
                                          ob[:])
    return nc


_cache = {}


def _get_kernels(reps=(1, 1)):
    key = ("p", reps)
    if key not in _cache:
        _install_patch()
        _cache[key] = (_build_phase1(reps[0]), _build_phase2(reps[1]))
    return _cache[key]


# ------------------------------------------------------------- host wrapper
def kernel(x, Wq, Wk, Wv, Wo, mixer_W, mixer_beta, freqs_cos, freqs_sin,
           _trace=False, _reps=(1, 1)):
    from concourse.bass_utils import run_bass_kernel_spmd

    x = np.asarray(x, np.float32)
    nc1, nc2 = _get_kernels(_reps)

    # ---- host prep, phase 1
    xT = np.ascontiguousarray(x[0].T)                       # [C, T] f32
    # [NTB, 128, 32*TBLK] : [tb, p, c*TBLK + t] = xT[c*128+p, tb*TBLK+t]
    xt_host = np.ascontiguousarray(
        xT.reshape(32, 128, NTB, TBLK).transpose(2, 1, 0, 3)
        .reshape(NTB, 128, 32 * TBLK)).astype(BF)

    perm = np.concatenate([np.arange(0, 128, 2), np.arange(1, 128, 2)])
    colperm = np.concatenate([h * 128 + perm for h in range(H)])

    weq = _weff(np.asarray(Wq, np.float32))[:, colperm]
    wek = _weff(np.asarray(Wk, np.float32))[:, colperm]
    wev = _weff(np.asarray(Wv, np.float32))

    def qk_layout(w):  # [4096, 512] -> [512, 4096] strips (colb*128+p, c*128+m)
        return np.ascontiguousarray(
            w.reshape(32, 128, 4, 128).transpose(2, 1, 0, 3).reshape(512, 4096)
        ).astype(BF)

    def v_layout(w):   # [4096, 512] -> [128, 32*512]
        return np.ascontiguousarray(
            w.reshape(32, 128, 512).transpose(1, 0, 2).reshape(128, 32 * 512)
        ).astype(BF)

    csT = np.asarray(freqs_cos, np.float32).T               # [64, T]
    snT = np.asarray(freqs_sin, np.float32).T
    cs_host = np.ascontiguousarray(np.concatenate([csT, csT], 0))        # [128,T]
    sn_host = np.ascontiguousarray(np.concatenate([-snT, snT], 0))

    f = np.arange(128)[None, :]
    p = np.arange(128)[:, None]
    tri_host = (f >= p).astype(np.float32).astype(BF)       # [128, 128]

    in_maps1 = []
    for c in range(NC):
        sl = slice(c * CPB, (c + 1) * CPB)
        in_maps1.append(dict(
            xt=xt_host,
            wq=qk_layout(weq[:, sl]),
            wk=qk_layout(wek[:, sl]),
            wv=v_layout(wev[:, sl]),
            cs=cs_host, sn=sn_host, tri=tri_host,
        ))

    r1 = run_bass_kernel_spmd(nc1, in_maps1, core_ids=list(range(NC)),
                              trace=_trace)
    yT = np.concatenate([np.asarray(r1.results[c]["yt"], np.float32)
                         for c in range(NC)], 0)            # [C, T] f32
    dd = np.concatenate([np.asarray(r1.results[c]["dd"], np.float32)
                         .reshape(4, T) for c in range(NC)], 0)  # [H, T] f32

    # ---- host: normalize y by softmax denominators, fold mixer into Wo
    yn = yT.reshape(H, D, T) / dd[:, None, :]
    yt2_host = np.ascontiguousarray(yn.astype(BF))          # [32, 128, T]

    beta = np.asarray(mixer_beta, np.float32)
    mw = np.asarray(mixer_W, np.float32)
    Mg = np.empty((1024, 1024), np.float32)     # [(j,d), (i,e)]
    for i in range(8):
        for j in range(8):
            Mg[j * 128:(j + 1) * 128, i * 128:(i + 1) * 128] = \
                (_SIGN[i, j] * mw[i ^ j]) * beta[None, :]

    weo = _weff(np.asarray(Wo, np.float32))                 # [4096, 4096]
    wfused = np.empty((C, C), np.float32)
    for g in range(4):
        rows = slice(g * 1024, (g + 1) * 1024)
        wfused[rows] = Mg @ weo[rows]

    in_maps2 = []
    for c in range(NC):
        wslice = wfused[:, c * 512:(c + 1) * 512]
        in_maps2.append(dict(yt2=yt2_host, wf=v_layout(wslice)))

    r2 = run_bass_kernel_spmd(nc2, in_maps2, core_ids=list(range(NC)),
                              trace=_trace)
    out = np.concatenate([np.asarray(r2.results[c]["out"], np.float32)
                          for c in range(NC)], 1)          # [T, C]
    out = np.ascontiguousarray(out)
    return (out.reshape(1, T, C).astype(np.float32), (r1, r2)) if _trace \
        else out.reshape(1, T, C).astype(np.float32)
